# revision 36
# baseline (speedup 1.0000x reference)
"""BENDR contrastive-loss kernel for Trainium2 (8 NeuronCores).

Reference computation (see problem): for each (b, t):
  logits[b*T+t, 0]   = cos(z[b,:,t], c[b,:,t+1]) / TEMP
  logits[b*T+t, 1+k] = cos(z[b,:,t], z[b,:,n(b,t,k)]) / TEMP
with n(b,t,k) = negative_inds[b, t*K+k] (row-local), TEMP=0.5.

Strategy: data-parallel over batch (2 rows per core).  Every negative logit
is an entry of the symmetric Gram matrix G = z^T z (z columns = feature
vectors), scaled by 2/(|z_t||z_j|); the norms are G's own diagonal.  So the
device only computes, per batch row:
  - the UPPER-TRIANGLE 128-row blocks of G (raw bf16 z, f32 PSUM) -> fp16
    (tau-th block covers columns [128*tau, T), so ~half the matmuls and
    traffic of the full Gram; the host mirrors lower-triangle lookups),
  - u[t]   = sum_f z[f,t]*c[f,t]   (DVE mult + ones-matmul reduction),
  - nc2[t] = sum_f c[f,t]^2        (same),
shipped as one [1, T] f32 DMA straight out of PSUM partition 0.
The host (pure indexing + O(output) normalize, same spirit as the
baseline's host gather) forms
  neg = 2*G[t,n] / sqrt(G[t,t]*G[n,n]),  pos = 2*u[t] / sqrt(G[t,t]*nc2[t]).

vs. the previous full-Gram kernel this removes the entire on-device
normalization pipeline (reciprocal 62us, input casts, scale mults, scaled
copies) whose DVE/ACT bursts head-blocked PSUM evacuation and let the PE's
HAM clock-gate throttle it to 1.2 GHz.  Here DVE/ACT only carry light
elementwise work + evacuation, and the PE stream is dense.

The gather itself stays on host: GPSIMD indirect_copy measures ~29us per
1024 indices and indirect DMA ~62ns/row -- computing the Gram block on the
PE and shipping fp16 is far cheaper than any on-device gather.
"""

import sys

for _p in ("/opt/trn_rl_repo",):
    if _p not in sys.path:
        sys.path.append(_p)

import numpy as np
import ml_dtypes

import concourse.bass as bass
import concourse.mybir as mybir
from concourse import tile as _tile
from concourse.tile import TileContext
from concourse.bass_utils import run_bass_kernel_spmd

dt = mybir.dt


B, F, T, K = 16, 256, 2048, 20
NCORES = 8
ROWS = B // NCORES          # batch rows per core
NBLK = T // 128             # t-blocks per batch row
FCH = F // 128              # f chunks (partition dim)
EPS = 1e-8

# ---------------------------------------------------------------------------
# Walrus in this container rejects instructions that carry more than one
# semaphore wait ("Too many sync wait commands").  Two shims fix that: the
# tile tail drain gets its waits on single-wait NOPs, and a post-pass splits
# any remaining multi-wait instruction.
# ---------------------------------------------------------------------------


def _patched_drain_and_barrier(self, tick_clock, wait_clock):
    nop0 = self.nc.sync.nop(nofuse=True, hint="tail_wait")
    wait_clock.add_sem_waits(
        nop0.ins, _tile.ScopedClock({None: tick_clock.global_clock})
    )
    si = nop0.ins.sync_info
    if si is not None and len(si.on_wait) > 1:
        waits = list(si.on_wait)
        nop0.ins.sync_info = mybir.SyncInfo(
            on_wait=waits[:1], on_update=list(si.on_update)
        )
        for w in waits[1:]:
            nopi = self.nc.sync.nop(nofuse=True, hint="tail_wait")
            nopi.ins.sync_info = mybir.SyncInfo(on_wait=[w], on_update=[])
    self.nc.sync.drain()
    self.nc.all_engine_barrier()
    assert self.sems is not None
    popped = self.nc._tile_sem_poison_stack.pop()
    assert popped is self._sem_poison
    self.nc.clear_and_free_semaphores(list(self.sems.allocated().values()))
    self.nc.all_engine_barrier()


_tile.TileContext._drain_and_barrier = _patched_drain_and_barrier

_wnop_counter = [0]


def split_excess_waits(nc, cap=1):
    for f in nc.m.functions:
        for bb in f.blocks:
            insts = bb.instructions
            out = []
            changed = False
            for inst in list(insts):
                si = getattr(inst, "sync_info", None)
                waits = list(si.on_wait) if si is not None else []
                if len(waits) > cap:
                    keep = waits[-cap:]
                    for w in waits[: len(waits) - cap]:
                        _wnop_counter[0] += 1
                        nop = mybir.InstNoOp(
                            name=f"wnop-{_wnop_counter[0]}", ins=[], outs=[]
                        )
                        nop.engine = inst.engine
                        nop.sync_info = mybir.SyncInfo(on_wait=[w], on_update=[])
                        out.append(nop)
                    inst.sync_info = mybir.SyncInfo(
                        on_wait=keep, on_update=list(si.on_update)
                    )
                    changed = True
                out.append(inst)
            if changed:
                insts[:] = out


def dedup_ldweights(nc):
    """The tile lowering emits an explicit InstLdweights before every
    InstMatmult.  Consecutive matmuls that share the stationary operand
    (same AP + tile position) don't need the reload -- the PE keeps its
    weights.  Convert redundant loads into NoOps (keeping their sync info)."""
    n = 0
    for f in nc.m.functions:
        for bb in f.blocks:
            insts = bb.instructions
            last_key = None
            out = []
            changed = False
            for inst in list(insts):
                tn = type(inst).__name__
                if tn == "InstLdweights":
                    key = (
                        str(inst.ins[0]),
                        tuple(inst.tile_position or ()),
                        tuple(inst.tile_size or ()),
                        bool(inst.is_transpose),
                    )
                    if key == last_key:
                        nop = mybir.InstNoOp(name=f"ldwnop-{n}", ins=[], outs=[])
                        n += 1
                        nop.engine = inst.engine
                        si = inst.sync_info
                        if si is not None:
                            nop.sync_info = mybir.SyncInfo(
                                on_wait=list(si.on_wait), on_update=list(si.on_update)
                            )
                        out.append(nop)
                        changed = True
                        continue
                    last_key = key
                elif tn == "InstMatmult":
                    if inst.is_transpose:
                        last_key = None
                out.append(inst)
            if changed:
                insts[:] = out
    return n


# ---------------------------------------------------------------------------
# Device program
# ---------------------------------------------------------------------------


def build_program():
    nc = bass.Bass("TRN2", num_devices=NCORES)
    # z8[r, p, ko, t] = z[r, ko*128 + p, t] as fp8 e4m3 -- the layout the
    # DoubleRow matmul wants ([K=128 partitions, Ko=2, free]).
    z8_in = nc.dram_tensor(
        "z8", [ROWS, 128, FCH, T], dt.float8e4, kind="ExternalInput"
    )
    z_in = nc.dram_tensor(
        "z", [ROWS, 128, FCH, T], dt.bfloat16, kind="ExternalInput"
    )
    c_in = nc.dram_tensor(
        "c", [ROWS, 128, FCH, T], dt.bfloat16, kind="ExternalInput"
    )
    # upper-triangle Gram blocks, PARTITION-MAJOR: g[p, r*NBLK+tau, j] =
    # G[128*tau + p, j] (valid for j >= 128*tau).  This layout lets one 3D
    # DMA ship TWO consecutive tau blocks (dims p, tau, j match the SBUF
    # enumeration order), halving the ~700ns-per-DMA trigger cost.
    g_out = nc.dram_tensor(
        "g", [128, ROWS * NBLK, T], dt.float16, kind="ExternalOutput"
    )
    # stat[2*r + 0, :] = u (z.c dot), stat[2*r + 1, :] = |c|^2
    stat_out = nc.dram_tensor(
        "stat", [ROWS * 2, T], dt.float32, kind="ExternalOutput"
    )

    with TileContext(nc) as tc:
        with (
            tc.tile_pool(name="io", bufs=2) as io_pool,
            tc.tile_pool(name="work", bufs=2) as work,
            tc.tile_pool(name="outp", bufs=1) as outp,
            tc.tile_pool(name="gram_ps", bufs=6, space="PSUM") as gram_ps,
            tc.tile_pool(name="stat_ps", bufs=2, space="PSUM") as stat_ps,
        ):
            ones16 = io_pool.tile([128, 128], dt.bfloat16, name="ones16")
            nc.vector.memset(ones16[:], 1.0)

            tiles = {}

            def emit_loads(r, eng):
                # Row 0's loads trigger from the GPSIMD queue (its preamble
                # ends first); row 1's from sync, so they don't delay row
                # 0's u-multiplies in the GPSIMD FIFO.  z8 loads in 512-col
                # quarters so tau 0's first matmul starts ~1us earlier.
                z8 = io_pool.tile([128, FCH, T], dt.float8e4, name="z8", tag="z8")
                for h in range(4):
                    sl = slice(512 * h, 512 * (h + 1))
                    eng.dma_start(out=z8[:, :, sl], in_=z8_in[r, :, :, sl])
                z16 = io_pool.tile([128, FCH, T], dt.bfloat16, name="z16", tag="z16")
                eng.dma_start(out=z16[:], in_=z_in[r])
                c16 = io_pool.tile([128, FCH, T], dt.bfloat16, name="c16", tag="c16")
                eng.dma_start(out=c16[:], in_=c_in[r])
                tiles[r] = (z8, z16, c16)

            def make_stats(r):
                """Closures for row r's stat work.  u-multiplies run on the
                idle GPSIMD (SBUF-only), csq on DVE; reduce matmuls + tiny
                stage copies are placed at pair boundaries late enough that
                the PE never waits on the elementwise producers."""
                _, z16, c16 = tiles[r]
                ut = work.tile([128, FCH, T], dt.bfloat16, name="u", tag=f"u{r}")
                stat_sb = [
                    work.tile([1, T], dt.float32, name=f"st{i}", tag=f"st{r}{i}")
                    for i in range(2)
                ]

                def muls():  # u_j = z_j * c_j  (GPSIMD, ~3.8us each)
                    for j in range(FCH):
                        nc.gpsimd.tensor_tensor(
                            out=ut[:, j, :], in0=z16[:, j, :], in1=c16[:, j, :],
                            op=mybir.AluOpType.mult,
                        )

                def csqs():  # c_j *= c_j in place (DVE; WAR on gpsimd u-muls)
                    for j in range(FCH):
                        nc.vector.tensor_tensor(
                            out=c16[:, j, :], in0=c16[:, j, :], in1=c16[:, j, :],
                            op=mybir.AluOpType.mult,
                        )

                def reduce(stat_idx, quarter):
                    # ones-matmul partition reduction into PSUM (sums
                    # replicated on every partition); stage row 0 to SBUF
                    # (DMA cannot read PSUM), DMA once the row is complete.
                    src = ut if stat_idx == 0 else c16
                    sl = slice(512 * quarter, 512 * (quarter + 1))
                    ps = stat_ps.tile([128, 512], dt.float32, name="sps", tag="sps")
                    for j in range(FCH):
                        nc.tensor.matmul(
                            ps[:], ones16[:], src[:, j, sl],
                            start=(j == 0), stop=(j == FCH - 1),
                        )
                    if (stat_idx * 4 + quarter) % 2 == 0:
                        nc.scalar.copy(stat_sb[stat_idx][0:1, sl], ps[0:1, :])
                    else:
                        nc.vector.tensor_copy(stat_sb[stat_idx][0:1, sl], ps[0:1, :])
                    if quarter == 3:
                        # sync queue, NOT gpsimd: a dependency-gated trigger
                        # would head-block gpsimd's multiply FIFO for ~20us.
                        row = 2 * r + stat_idx
                        nc.sync.dma_start(
                            out=stat_out[row : row + 1, :],
                            in_=stat_sb[stat_idx][0:1, :],
                        )

                return muls, csqs, reduce

            # manual ring of 6 pair-otiles ([t-block 2k | t-block 2k+1]; the
            # second block is left-padded 128 junk cols so one 3D DMA covers
            # both blocks with a single column base).  6 deep because the
            # early pair DMAs are ~2MB / ~5us: with only 3 slots the
            # evacuation (and then the PE, via the PSUM ring) stalls on the
            # write-after-read of a slot still being shipped out.
            NOR = 6
            oring = [
                outp.tile([128, 2, T], dt.float16, name=f"ot{i}", tag=f"ot{i}")
                for i in range(NOR)
            ]
            evac_flip = [0]

            def emit_gram_tau(r, tau, ot, ko):
                """Matmuls + PSUM evacuation for one tau block into half `ko`
                of the pair otile `ot` (left-padded 128 cols when ko=1)."""
                z8 = tiles[r][0]
                t0 = 128 * tau
                w = T - t0
                nch = (w + 511) // 512
                lhsT = z8[:, :, t0 : t0 + 128]
                pts = []
                for c in range(nch):
                    pts.append(
                        gram_ps.tile([128, 512], dt.float32, name="gps", tag="gps")
                    )
                for c in range(nch):
                    cw = min(512, w - 512 * c)
                    c0 = t0 + 512 * c
                    # fp8 DoubleRow: [128, Ko=2, free] operands; full 256-deep
                    # contraction in one pass, 2 MACs/cell/cycle.
                    nc.tensor.matmul(
                        pts[c][:, :cw], lhsT, z8[:, :, c0 : c0 + cw],
                        start=True, stop=True,
                        perf_mode=mybir.MatmulPerfMode.DoubleRow,
                    )
                pad = 128 * ko
                for c in range(nch):
                    cw = min(512, w - 512 * c)
                    dst = ot[:, ko, pad + 512 * c : pad + 512 * c + cw]
                    if evac_flip[0] % 2 == 0:
                        nc.scalar.copy(dst, pts[c][:, :cw])
                    else:
                        nc.vector.tensor_copy(dst, pts[c][:, :cw])
                    evac_flip[0] += 1

            emit_loads(0, nc.gpsimd)
            emit_loads(1, nc.sync)
            stats = {r: make_stats(r) for r in range(ROWS)}
            # Global schedule keyed by pair index 0..15 (8 pairs per row):
            # each entry runs after that pair's DMA is emitted.  Producers
            # (muls ~7.6us on gpsimd, csqs ~2.2us on DVE) are emitted early;
            # the PE-side reduces land boundaries after the producer's
            # estimated completion so the in-order PE queue never waits.
            sched = {
                1: [stats[0][0], stats[1][0]],                # both rows' u-muls
                5: [lambda: stats[0][2](0, 0), lambda: stats[0][2](0, 1)],
                6: [lambda: stats[0][2](0, 2), lambda: stats[0][2](0, 3),
                    stats[0][1]],                             # r0 csq (DVE)
                8: [lambda: stats[0][2](1, 0), lambda: stats[0][2](1, 1)],
                9: [lambda: stats[0][2](1, 2), lambda: stats[0][2](1, 3)],
                11: [lambda: stats[1][2](0, 0), lambda: stats[1][2](0, 1)],
                12: [lambda: stats[1][2](0, 2), lambda: stats[1][2](0, 3),
                     stats[1][1]],                            # r1 csq (DVE)
                14: [lambda: stats[1][2](1, 0), lambda: stats[1][2](1, 1)],
                15: [lambda: stats[1][2](1, 2), lambda: stats[1][2](1, 3)],
            }
            for r in range(ROWS):
                sid = nc.enter_named_scope(f"gram_r{r}", False)[0]
                for pair in range(NBLK // 2):
                    gp = r * (NBLK // 2) + pair
                    ot = oring[gp % NOR]
                    emit_gram_tau(r, 2 * pair, ot, 0)
                    emit_gram_tau(r, 2 * pair + 1, ot, 1)
                    wa = T - 256 * pair
                    blk = r * NBLK + 2 * pair
                    nc.sync.dma_start(
                        out=g_out[:, blk : blk + 2, 256 * pair :],
                        in_=ot[:, :, :wa],
                    )
                    for fn in sched.get(gp, []):
                        fn()
                nc.leave_named_scope(f"gram_r{r}", sid, False)

    dedup_ldweights(nc)
    split_excess_waits(nc)
    return nc


_PROGRAM = None


def _get_program():
    global _PROGRAM
    if _PROGRAM is None:
        _PROGRAM = build_program()
    return _PROGRAM


def kernel(z, c, negative_inds, _trace=False):
    z = np.asarray(z)
    c = np.asarray(c)
    ni = np.asarray(negative_inds)
    assert z.shape == (B, F, T) and c.shape == (B, F, T + 1)

    # [B, 128, FCH, T]: x[b, p, j, t] = x[b, j*128+p, t] -- the partition-
    # major layout every SBUF tile uses (and DoubleRow wants for z8).
    zt = z.reshape(B, FCH, 128, T).transpose(0, 2, 1, 3)
    z16 = np.ascontiguousarray(zt.astype(ml_dtypes.bfloat16))
    z8 = np.ascontiguousarray(zt.astype(ml_dtypes.float8_e4m3fn))
    c16 = np.ascontiguousarray(
        c[:, :, 1:].reshape(B, FCH, 128, T).transpose(0, 2, 1, 3).astype(
            ml_dtypes.bfloat16
        )
    )

    nc = _get_program()
    in_maps = []
    for core in range(NCORES):
        rs = slice(core * ROWS, (core + 1) * ROWS)
        in_maps.append({"z8": z8[rs], "z": z16[rs], "c": c16[rs]})

    res = run_bass_kernel_spmd(nc, in_maps, list(range(NCORES)), trace=_trace)

    # [B, T, T] fp16 raw Gram, upper-triangle blocks valid; [B, 2, T] stats.
    # g result arrives partition-major [128, ROWS*NBLK, T].
    g = np.concatenate(
        [
            res.results[i]["g"].transpose(1, 0, 2).reshape(ROWS, T, T)
            for i in range(NCORES)
        ],
        axis=0,
    )
    stat = np.concatenate(
        [res.results[i]["stat"].reshape(ROWS, 2, T) for i in range(NCORES)], axis=0
    )  # [B, 2, T]
    u = stat[:, 0, :].astype(np.float64)
    nc2 = stat[:, 1, :].astype(np.float64)

    # host-side unshard: mirror the triangle, normalize, gather (O(output))
    ti = np.arange(T)
    nz2 = np.ascontiguousarray(g[:, ti, ti]).astype(np.float64)  # [B, T] diag
    nz = np.sqrt(nz2)

    n = ni.reshape(B, T, K).astype(np.int64)
    tt = ti[None, :, None]
    valid = n >= (tt // 128) * 128
    rown = np.where(valid, tt, n)
    coln = np.where(valid, n, tt)
    bidx = np.arange(B)[:, None, None]
    graw = g[bidx, rown, coln].astype(np.float64)          # [B, T, K]
    denom = np.maximum(nz[bidx, tt] * nz[bidx, n], EPS)
    neg = (graw / denom) * 2.0

    pos = (u / np.maximum(nz * np.sqrt(nc2), EPS)) * 2.0   # [B, T]

    logits = np.concatenate([pos[:, :, None], neg], axis=2).astype(np.float32)
    out = logits.reshape(B * T, K + 1)
    if _trace:
        return out, res
    return out


if __name__ == "__main__":
    rng = np.random.default_rng(0)
    z = rng.standard_normal((B, F, T), dtype=np.float32)
    c = rng.standard_normal((B, F, T + 1), dtype=np.float32)
    ni = rng.integers(0, T - 1, size=(B, T * K)).astype(np.int64)
    out = kernel(z=z, c=c, negative_inds=ni)
    print("out", out.shape, out.dtype, np.isfinite(out).all())


# revision 37
# speedup vs baseline: 1.3773x; 1.3773x over previous
"""BENDR contrastive-loss kernel for Trainium2 (8 NeuronCores).

Reference computation (see problem): for each (b, t):
  logits[b*T+t, 0]   = cos(z[b,:,t], c[b,:,t+1]) / TEMP
  logits[b*T+t, 1+k] = cos(z[b,:,t], z[b,:,n(b,t,k)]) / TEMP
with n(b,t,k) = negative_inds[b, t*K+k] (row-local), TEMP=0.5.

Strategy: data-parallel over batch (2 rows per core).  Every negative logit
is an entry of the symmetric Gram matrix G = z^T z (z columns = feature
vectors) scaled by 2/(|z_t||z_j|), and the norms are G's own diagonal.  The
device is a PURE Gram kernel: it computes the UPPER-TRIANGLE 128-row blocks
of G in fp8-e4m3 with DoubleRow matmuls (256-deep contraction in one pass,
2 MACs/cell/cycle) and ships them as fp16.  Everything O(B*T*F) or smaller
-- the positive column 2*u[t]/(|z_t||c_t|), the c-norms, the normalize and
the index-pick gather -- runs on the host (0.1% of the FLOPs; numpy).

Normalizing the negatives by the fp8 Gram's own diagonal is what makes fp8
viable: logits become exact cosines of the QUANTIZED vectors, so the
correlated quantization error cancels (measured rel-err 1.16e-2 against the
2e-2 gate; with exact norms instead it fails at 2.2e-2).  Self-hits
(n == t) become exactly 2.0 automatically.

Per-core timeline: z8 loads in 512-col chunks (GPSIMD-issued triggers, the
sync queue carries the 16 output DMAs); tau pairs stream through a 6-deep
PSUM ring, PSUM->SBUF fp16 evacuation alternates DVE/ACT; two consecutive
tau blocks share one 3D pair-DMA (the second block left-padded 128 junk
cols, never read by the host) into a partition-major DRAM layout.

On-device gathers were measured and rejected: GPSIMD indirect_copy ~29us
per 1024 indices, indirect DMA ~62ns/row -- computing the full Gram block
on the PE and shipping fp16 is far cheaper.
"""

import sys

for _p in ("/opt/trn_rl_repo",):
    if _p not in sys.path:
        sys.path.append(_p)

import numpy as np
import ml_dtypes

import concourse.bass as bass
import concourse.mybir as mybir
from concourse import tile as _tile
from concourse.tile import TileContext
from concourse.bass_utils import run_bass_kernel_spmd

dt = mybir.dt


B, F, T, K = 16, 256, 2048, 20
NCORES = 8
ROWS = B // NCORES          # batch rows per core
NBLK = T // 128             # t-blocks per batch row
FCH = F // 128              # f chunks (partition dim)
EPS = 1e-8

# ---------------------------------------------------------------------------
# Walrus in this container rejects instructions that carry more than one
# semaphore wait ("Too many sync wait commands").  Two shims fix that: the
# tile tail drain gets its waits on single-wait NOPs, and a post-pass splits
# any remaining multi-wait instruction.
# ---------------------------------------------------------------------------


def _patched_drain_and_barrier(self, tick_clock, wait_clock):
    nop0 = self.nc.sync.nop(nofuse=True, hint="tail_wait")
    wait_clock.add_sem_waits(
        nop0.ins, _tile.ScopedClock({None: tick_clock.global_clock})
    )
    si = nop0.ins.sync_info
    if si is not None and len(si.on_wait) > 1:
        waits = list(si.on_wait)
        nop0.ins.sync_info = mybir.SyncInfo(
            on_wait=waits[:1], on_update=list(si.on_update)
        )
        for w in waits[1:]:
            nopi = self.nc.sync.nop(nofuse=True, hint="tail_wait")
            nopi.ins.sync_info = mybir.SyncInfo(on_wait=[w], on_update=[])
    self.nc.sync.drain()
    self.nc.all_engine_barrier()
    assert self.sems is not None
    popped = self.nc._tile_sem_poison_stack.pop()
    assert popped is self._sem_poison
    self.nc.clear_and_free_semaphores(list(self.sems.allocated().values()))
    self.nc.all_engine_barrier()


_tile.TileContext._drain_and_barrier = _patched_drain_and_barrier

_wnop_counter = [0]


def split_excess_waits(nc, cap=1):
    for f in nc.m.functions:
        for bb in f.blocks:
            insts = bb.instructions
            out = []
            changed = False
            for inst in list(insts):
                si = getattr(inst, "sync_info", None)
                waits = list(si.on_wait) if si is not None else []
                if len(waits) > cap:
                    keep = waits[-cap:]
                    for w in waits[: len(waits) - cap]:
                        _wnop_counter[0] += 1
                        nop = mybir.InstNoOp(
                            name=f"wnop-{_wnop_counter[0]}", ins=[], outs=[]
                        )
                        nop.engine = inst.engine
                        nop.sync_info = mybir.SyncInfo(on_wait=[w], on_update=[])
                        out.append(nop)
                    inst.sync_info = mybir.SyncInfo(
                        on_wait=keep, on_update=list(si.on_update)
                    )
                    changed = True
                out.append(inst)
            if changed:
                insts[:] = out


def dedup_ldweights(nc):
    """The tile lowering emits an explicit InstLdweights before every
    InstMatmult.  Consecutive matmuls that share the stationary operand
    (same AP + tile position) don't need the reload -- the PE keeps its
    weights.  Convert redundant loads into NoOps (keeping their sync info)."""
    n = 0
    for f in nc.m.functions:
        for bb in f.blocks:
            insts = bb.instructions
            last_key = None
            out = []
            changed = False
            for inst in list(insts):
                tn = type(inst).__name__
                if tn == "InstLdweights":
                    key = (
                        str(inst.ins[0]),
                        tuple(inst.tile_position or ()),
                        tuple(inst.tile_size or ()),
                        bool(inst.is_transpose),
                    )
                    if key == last_key:
                        nop = mybir.InstNoOp(name=f"ldwnop-{n}", ins=[], outs=[])
                        n += 1
                        nop.engine = inst.engine
                        si = inst.sync_info
                        if si is not None:
                            nop.sync_info = mybir.SyncInfo(
                                on_wait=list(si.on_wait), on_update=list(si.on_update)
                            )
                        out.append(nop)
                        changed = True
                        continue
                    last_key = key
                elif tn == "InstMatmult":
                    if inst.is_transpose:
                        last_key = None
                out.append(inst)
            if changed:
                insts[:] = out
    return n


# ---------------------------------------------------------------------------
# Device program: pure fp8 upper-triangle Gram
# ---------------------------------------------------------------------------


def build_program():
    nc = bass.Bass("TRN2", num_devices=NCORES)
    # z8[r, p, ko, t] = z[r, ko*128 + p, t] as fp8 e4m3 -- the layout the
    # DoubleRow matmul wants ([K=128 partitions, Ko=2, free]).
    z8_in = nc.dram_tensor(
        "z8", [ROWS, 128, FCH, T], dt.float8e4, kind="ExternalInput"
    )
    # upper-triangle Gram blocks, PARTITION-MAJOR: g[p, r*NBLK+tau, j] =
    # G[128*tau + p, j] (valid for j >= 128*tau).  This layout lets one 3D
    # DMA ship TWO consecutive tau blocks (dims p, tau, j match the SBUF
    # enumeration order), halving the ~700ns-per-DMA trigger cost.
    g_out = nc.dram_tensor(
        "g", [128, ROWS * NBLK, T], dt.float16, kind="ExternalOutput"
    )

    with TileContext(nc) as tc:
        with (
            tc.tile_pool(name="io", bufs=2) as io_pool,
            tc.tile_pool(name="outp", bufs=1) as outp,
            tc.tile_pool(name="gram_ps", bufs=6, space="PSUM") as gram_ps,
        ):
            tiles = {}

            def emit_loads(r, nchunk):
                # input triggers ride the (otherwise idle) GPSIMD queue; z8
                # in column chunks so tau 0 starts on the first chunk.
                z8 = io_pool.tile([128, FCH, T], dt.float8e4, name="z8", tag="z8")
                step = T // nchunk
                for h in range(nchunk):
                    sl = slice(step * h, step * (h + 1))
                    nc.gpsimd.dma_start(out=z8[:, :, sl], in_=z8_in[r, :, :, sl])
                tiles[r] = z8

            # manual ring of 6 pair-otiles ([t-block 2k | t-block 2k+1]; the
            # second block is left-padded 128 junk cols so one 3D DMA covers
            # both blocks with a single column base).  6 deep because the
            # early pair DMAs are ~2MB / ~5us: with fewer slots the
            # evacuation (and then the PE, via the PSUM ring) stalls on the
            # write-after-read of a slot still being shipped out.
            NOR = 6
            oring = [
                outp.tile([128, 2, T], dt.float16, name=f"ot{i}", tag=f"ot{i}")
                for i in range(NOR)
            ]
            evac_flip = [0]

            def emit_gram_tau(r, tau, ot, ko):
                """Matmuls + PSUM evacuation for one tau block into half `ko`
                of the pair otile `ot` (left-padded 128 cols when ko=1)."""
                z8 = tiles[r]
                t0 = 128 * tau
                w = T - t0
                nch = (w + 511) // 512
                lhsT = z8[:, :, t0 : t0 + 128]
                pts = []
                for c in range(nch):
                    pts.append(
                        gram_ps.tile([128, 512], dt.float32, name="gps", tag="gps")
                    )
                for c in range(nch):
                    cw = min(512, w - 512 * c)
                    c0 = t0 + 512 * c
                    nc.tensor.matmul(
                        pts[c][:, :cw], lhsT, z8[:, :, c0 : c0 + cw],
                        start=True, stop=True,
                        perf_mode=mybir.MatmulPerfMode.DoubleRow,
                    )
                pad = 128 * ko
                for c in range(nch):
                    cw = min(512, w - 512 * c)
                    dst = ot[:, ko, pad + 512 * c : pad + 512 * c + cw]
                    # DVE's PSUM->fp16 cast measures ~504ns vs ACT's ~590ns
                    # per 512 cols: give DVE 6 of every 11 chunks.
                    if (evac_flip[0] * 6) % 11 < 6:
                        nc.vector.tensor_copy(dst, pts[c][:, :cw])
                    else:
                        nc.scalar.copy(dst, pts[c][:, :cw])
                    evac_flip[0] += 1

            emit_loads(0, 4)
            emit_loads(1, 2)
            for r in range(ROWS):
                sid = nc.enter_named_scope(f"gram_r{r}", False)[0]
                for pair in range(NBLK // 2):
                    gp = r * (NBLK // 2) + pair
                    ot = oring[gp % NOR]
                    emit_gram_tau(r, 2 * pair, ot, 0)
                    emit_gram_tau(r, 2 * pair + 1, ot, 1)
                    wa = T - 256 * pair
                    blk = r * NBLK + 2 * pair
                    nc.sync.dma_start(
                        out=g_out[:, blk : blk + 2, 256 * pair :],
                        in_=ot[:, :, :wa],
                    )
                nc.leave_named_scope(f"gram_r{r}", sid, False)

    dedup_ldweights(nc)
    split_excess_waits(nc)
    return nc


_PROGRAM = None


def _get_program():
    global _PROGRAM
    if _PROGRAM is None:
        _PROGRAM = build_program()
    return _PROGRAM


def kernel(z, c, negative_inds, _trace=False):
    z = np.asarray(z)
    c = np.asarray(c)
    ni = np.asarray(negative_inds)
    assert z.shape == (B, F, T) and c.shape == (B, F, T + 1)

    # [B, 128, FCH, T]: z8[b, p, j, t] = z[b, j*128+p, t] (DoubleRow layout)
    z8 = np.ascontiguousarray(
        z.reshape(B, FCH, 128, T).transpose(0, 2, 1, 3).astype(
            ml_dtypes.float8_e4m3fn
        )
    )

    nc = _get_program()
    in_maps = []
    for core in range(NCORES):
        rs = slice(core * ROWS, (core + 1) * ROWS)
        in_maps.append({"z8": z8[rs]})

    res = run_bass_kernel_spmd(nc, in_maps, list(range(NCORES)), trace=_trace)

    # [B, T, T] fp16 raw fp8-Gram, upper-triangle blocks valid (the result
    # arrives partition-major [128, ROWS*NBLK, T]).
    g = np.concatenate(
        [
            res.results[i]["g"].transpose(1, 0, 2).reshape(ROWS, T, T)
            for i in range(NCORES)
        ],
        axis=0,
    )

    # ---- host epilogue: O(B*T*F) stats + O(output) normalize/gather ----
    ti = np.arange(T)
    nz2 = np.ascontiguousarray(g[:, ti, ti]).astype(np.float64)  # fp8 diag
    nz = np.sqrt(nz2)

    n = ni.reshape(B, T, K).astype(np.int64)
    tt = ti[None, :, None]
    valid = n >= (tt // 128) * 128
    rown = np.where(valid, tt, n)
    coln = np.where(valid, n, tt)
    bidx = np.arange(B)[:, None, None]
    graw = g[bidx, rown, coln].astype(np.float64)          # [B, T, K]
    denom = np.maximum(nz[bidx, tt] * nz[bidx, n], EPS)
    neg = (graw / denom) * 2.0

    # positives: exact f32 math on the raw inputs (0.1% of the FLOPs)
    zf = z.astype(np.float64)
    cf = c[:, :, 1:].astype(np.float64)
    u = np.einsum("bft,bft->bt", zf, cf)
    pos_denom = np.maximum(
        np.sqrt((zf * zf).sum(axis=1) * (cf * cf).sum(axis=1)), EPS
    )
    pos = (u / pos_denom) * 2.0

    logits = np.concatenate([pos[:, :, None], neg], axis=2).astype(np.float32)
    out = logits.reshape(B * T, K + 1)
    if _trace:
        return out, res
    return out


if __name__ == "__main__":
    rng = np.random.default_rng(0)
    z = rng.standard_normal((B, F, T), dtype=np.float32)
    c = rng.standard_normal((B, F, T + 1), dtype=np.float32)
    ni = rng.integers(0, T - 1, size=(B, T * K)).astype(np.int64)
    out = kernel(z=z, c=c, negative_inds=ni)
    print("out", out.shape, out.dtype, np.isfinite(out).all())


# revision 38
# speedup vs baseline: 1.4934x; 1.0843x over previous
"""BENDR contrastive-loss kernel for Trainium2 (8 NeuronCores).

Reference computation (see problem): for each (b, t):
  logits[b*T+t, 0]   = cos(z[b,:,t], c[b,:,t+1]) / TEMP
  logits[b*T+t, 1+k] = cos(z[b,:,t], z[b,:,n(b,t,k)]) / TEMP
with n(b,t,k) = negative_inds[b, t*K+k] (row-local), TEMP=0.5.

Strategy: data-parallel over batch (2 rows per core).  Every negative logit
is an entry of the symmetric Gram matrix G = z^T z (z columns = feature
vectors) scaled by 2/(|z_t||z_j|), and the norms are G's own diagonal.  The
device is a PURE Gram kernel: it computes the UPPER-TRIANGLE 128-row blocks
of G in fp8-e4m3 with DoubleRow matmuls (256-deep contraction in one pass,
2 MACs/cell/cycle) and ships them as fp16.  Everything O(B*T*F) or smaller
-- the positive column 2*u[t]/(|z_t||c_t|), the c-norms, the normalize and
the index-pick gather -- runs on the host (0.1% of the FLOPs; numpy).

Normalizing the negatives by the fp8 Gram's own diagonal is what makes fp8
viable: logits become exact cosines of the QUANTIZED vectors, so the
correlated quantization error cancels (measured rel-err 1.16e-2 against the
2e-2 gate; with exact norms instead it fails at 2.2e-2).  Self-hits
(n == t) become exactly 2.0 automatically.

Per-core timeline: z8 loads in 512-col chunks (GPSIMD-issued triggers, the
sync queue carries the 16 output DMAs); tau pairs stream through a 6-deep
PSUM ring, PSUM->SBUF fp16 evacuation alternates DVE/ACT; two consecutive
tau blocks share one 3D pair-DMA (the second block left-padded 128 junk
cols, never read by the host) into a partition-major DRAM layout.

On-device gathers were measured and rejected: GPSIMD indirect_copy ~29us
per 1024 indices, indirect DMA ~62ns/row -- computing the full Gram block
on the PE and shipping fp16 is far cheaper.
"""

import sys

for _p in ("/opt/trn_rl_repo",):
    if _p not in sys.path:
        sys.path.append(_p)

import numpy as np
import ml_dtypes

import concourse.bass as bass
import concourse.mybir as mybir
from concourse import tile as _tile
from concourse.tile import TileContext
from concourse.bass_utils import run_bass_kernel_spmd

dt = mybir.dt


B, F, T, K = 16, 256, 2048, 20
NCORES = 8
ROWS = B // NCORES          # batch rows per core
NBLK = T // 128             # t-blocks per batch row
FCH = F // 128              # f chunks (partition dim)
EPS = 1e-8

# ---------------------------------------------------------------------------
# Walrus in this container rejects instructions that carry more than one
# semaphore wait ("Too many sync wait commands").  Two shims fix that: the
# tile tail drain gets its waits on single-wait NOPs, and a post-pass splits
# any remaining multi-wait instruction.
# ---------------------------------------------------------------------------


def _patched_drain_and_barrier(self, tick_clock, wait_clock):
    nop0 = self.nc.sync.nop(nofuse=True, hint="tail_wait")
    wait_clock.add_sem_waits(
        nop0.ins, _tile.ScopedClock({None: tick_clock.global_clock})
    )
    si = nop0.ins.sync_info
    if si is not None and len(si.on_wait) > 1:
        waits = list(si.on_wait)
        nop0.ins.sync_info = mybir.SyncInfo(
            on_wait=waits[:1], on_update=list(si.on_update)
        )
        for w in waits[1:]:
            nopi = self.nc.sync.nop(nofuse=True, hint="tail_wait")
            nopi.ins.sync_info = mybir.SyncInfo(on_wait=[w], on_update=[])
    self.nc.sync.drain()
    self.nc.all_engine_barrier()
    assert self.sems is not None
    popped = self.nc._tile_sem_poison_stack.pop()
    assert popped is self._sem_poison
    self.nc.clear_and_free_semaphores(list(self.sems.allocated().values()))
    self.nc.all_engine_barrier()


_tile.TileContext._drain_and_barrier = _patched_drain_and_barrier

_wnop_counter = [0]


def split_excess_waits(nc, cap=1):
    for f in nc.m.functions:
        for bb in f.blocks:
            insts = bb.instructions
            out = []
            changed = False
            for inst in list(insts):
                si = getattr(inst, "sync_info", None)
                waits = list(si.on_wait) if si is not None else []
                if len(waits) > cap:
                    keep = waits[-cap:]
                    for w in waits[: len(waits) - cap]:
                        _wnop_counter[0] += 1
                        nop = mybir.InstNoOp(
                            name=f"wnop-{_wnop_counter[0]}", ins=[], outs=[]
                        )
                        nop.engine = inst.engine
                        nop.sync_info = mybir.SyncInfo(on_wait=[w], on_update=[])
                        out.append(nop)
                    inst.sync_info = mybir.SyncInfo(
                        on_wait=keep, on_update=list(si.on_update)
                    )
                    changed = True
                out.append(inst)
            if changed:
                insts[:] = out


def dedup_ldweights(nc):
    """The tile lowering emits an explicit InstLdweights before every
    InstMatmult.  Consecutive matmuls that share the stationary operand
    (same AP + tile position) don't need the reload -- the PE keeps its
    weights.  Convert redundant loads into NoOps (keeping their sync info)."""
    n = 0
    for f in nc.m.functions:
        for bb in f.blocks:
            insts = bb.instructions
            last_key = None
            out = []
            changed = False
            for inst in list(insts):
                tn = type(inst).__name__
                if tn == "InstLdweights":
                    key = (
                        str(inst.ins[0]),
                        tuple(inst.tile_position or ()),
                        tuple(inst.tile_size or ()),
                        bool(inst.is_transpose),
                    )
                    if key == last_key:
                        nop = mybir.InstNoOp(name=f"ldwnop-{n}", ins=[], outs=[])
                        n += 1
                        nop.engine = inst.engine
                        si = inst.sync_info
                        if si is not None:
                            nop.sync_info = mybir.SyncInfo(
                                on_wait=list(si.on_wait), on_update=list(si.on_update)
                            )
                        out.append(nop)
                        changed = True
                        continue
                    last_key = key
                elif tn == "InstMatmult":
                    if inst.is_transpose:
                        last_key = None
                out.append(inst)
            if changed:
                insts[:] = out
    return n


# ---------------------------------------------------------------------------
# Device program: pure fp8 upper-triangle Gram
# ---------------------------------------------------------------------------


def build_program():
    nc = bass.Bass("TRN2", num_devices=NCORES)
    # z8[r, p, ko, t] = z[r, ko*128 + p, t] as fp8 e4m3 -- the layout the
    # DoubleRow matmul wants ([K=128 partitions, Ko=2, free]).
    z8_in = nc.dram_tensor(
        "z8", [ROWS, 128, FCH, T], dt.float8e4, kind="ExternalInput"
    )
    # upper-triangle Gram blocks, PARTITION-MAJOR: g[p, r*NBLK+tau, j] =
    # G[128*tau + p, j] (valid for j >= 128*tau).  This layout lets one 3D
    # DMA ship TWO consecutive tau blocks (dims p, tau, j match the SBUF
    # enumeration order), halving the ~700ns-per-DMA trigger cost.
    g_out = nc.dram_tensor(
        "g", [128, ROWS * NBLK, T], dt.float16, kind="ExternalOutput"
    )

    with TileContext(nc) as tc:
        with (
            tc.tile_pool(name="io", bufs=2) as io_pool,
            tc.tile_pool(name="outp", bufs=1) as outp,
            tc.tile_pool(name="gram_ps", bufs=6, space="PSUM") as gram_ps,
        ):
            tiles = {}

            def emit_loads(r, nchunk):
                # input triggers ride the (otherwise idle) GPSIMD queue; z8
                # in column chunks so tau 0 starts on the first chunk.
                z8 = io_pool.tile([128, FCH, T], dt.float8e4, name="z8", tag="z8")
                step = T // nchunk
                for h in range(nchunk):
                    sl = slice(step * h, step * (h + 1))
                    nc.gpsimd.dma_start(out=z8[:, :, sl], in_=z8_in[r, :, :, sl])
                tiles[r] = z8

            # manual ring of 6 pair-otiles ([t-block 2k | t-block 2k+1]; the
            # second block is left-padded 128 junk cols so one 3D DMA covers
            # both blocks with a single column base).  6 deep because the
            # early pair DMAs are ~2MB / ~5us: with fewer slots the
            # evacuation (and then the PE, via the PSUM ring) stalls on the
            # write-after-read of a slot still being shipped out.
            NOR = 6
            oring = [
                outp.tile([128, 2, T], dt.float16, name=f"ot{i}", tag=f"ot{i}")
                for i in range(NOR)
            ]
            evac_flip = [0]

            def emit_gram_tau(r, tau, ot, ko):
                """Matmuls + PSUM evacuation for one tau block into half `ko`
                of the pair otile `ot` (left-padded 128 cols when ko=1)."""
                z8 = tiles[r]
                t0 = 128 * tau
                w = T - t0
                nch = (w + 511) // 512
                lhsT = z8[:, :, t0 : t0 + 128]
                pts = []
                for c in range(nch):
                    pts.append(
                        gram_ps.tile([128, 512], dt.float32, name="gps", tag="gps")
                    )
                for c in range(nch):
                    cw = min(512, w - 512 * c)
                    c0 = t0 + 512 * c
                    nc.tensor.matmul(
                        pts[c][:, :cw], lhsT, z8[:, :, c0 : c0 + cw],
                        start=True, stop=True,
                        perf_mode=mybir.MatmulPerfMode.DoubleRow,
                    )
                pad = 128 * ko
                for c in range(nch):
                    cw = min(512, w - 512 * c)
                    dst = ot[:, ko, pad + 512 * c : pad + 512 * c + cw]
                    # DVE's PSUM->fp16 cast measures ~504ns vs ACT's ~590ns
                    # per 512 cols: give DVE 6 of every 11 chunks.
                    if (evac_flip[0] * 6) % 11 < 6:
                        nc.vector.tensor_copy(dst, pts[c][:, :cw])
                    else:
                        nc.scalar.copy(dst, pts[c][:, :cw])
                    evac_flip[0] += 1

            emit_loads(0, 4)
            emit_loads(1, 2)
            # Interleave the two rows' pairs, big-W first: row-serial order
            # emitted row 1's ~4.4MB of output in the last third of the
            # stream, leaving a ~10us DMA drain tail.  Row 1 trails by 3
            # slots so its z8 load (done ~15us) is ready when needed.
            order = []
            for i in range(NBLK // 2 + 3):
                if i < NBLK // 2:
                    order.append((0, i))
                if 0 <= i - 3 < NBLK // 2:
                    order.append((1, i - 3))
            for gp, (r, pair) in enumerate(order):
                ot = oring[gp % NOR]
                emit_gram_tau(r, 2 * pair, ot, 0)
                emit_gram_tau(r, 2 * pair + 1, ot, 1)
                wa = T - 256 * pair
                blk = r * NBLK + 2 * pair
                nc.sync.dma_start(
                    out=g_out[:, blk : blk + 2, 256 * pair :],
                    in_=ot[:, :, :wa],
                )

    dedup_ldweights(nc)
    split_excess_waits(nc)
    return nc


_PROGRAM = None


def _get_program():
    global _PROGRAM
    if _PROGRAM is None:
        _PROGRAM = build_program()
    return _PROGRAM


def kernel(z, c, negative_inds, _trace=False):
    z = np.asarray(z)
    c = np.asarray(c)
    ni = np.asarray(negative_inds)
    assert z.shape == (B, F, T) and c.shape == (B, F, T + 1)

    # [B, 128, FCH, T]: z8[b, p, j, t] = z[b, j*128+p, t] (DoubleRow layout)
    z8 = np.ascontiguousarray(
        z.reshape(B, FCH, 128, T).transpose(0, 2, 1, 3).astype(
            ml_dtypes.float8_e4m3fn
        )
    )

    nc = _get_program()
    in_maps = []
    for core in range(NCORES):
        rs = slice(core * ROWS, (core + 1) * ROWS)
        in_maps.append({"z8": z8[rs]})

    res = run_bass_kernel_spmd(nc, in_maps, list(range(NCORES)), trace=_trace)

    # [B, T, T] fp16 raw fp8-Gram, upper-triangle blocks valid (the result
    # arrives partition-major [128, ROWS*NBLK, T]).
    g = np.concatenate(
        [
            res.results[i]["g"].transpose(1, 0, 2).reshape(ROWS, T, T)
            for i in range(NCORES)
        ],
        axis=0,
    )

    # ---- host epilogue: O(B*T*F) stats + O(output) normalize/gather ----
    ti = np.arange(T)
    nz2 = np.ascontiguousarray(g[:, ti, ti]).astype(np.float64)  # fp8 diag
    nz = np.sqrt(nz2)

    n = ni.reshape(B, T, K).astype(np.int64)
    tt = ti[None, :, None]
    valid = n >= (tt // 128) * 128
    rown = np.where(valid, tt, n)
    coln = np.where(valid, n, tt)
    bidx = np.arange(B)[:, None, None]
    graw = g[bidx, rown, coln].astype(np.float64)          # [B, T, K]
    denom = np.maximum(nz[bidx, tt] * nz[bidx, n], EPS)
    neg = (graw / denom) * 2.0

    # positives: exact f32 math on the raw inputs (0.1% of the FLOPs)
    zf = z.astype(np.float64)
    cf = c[:, :, 1:].astype(np.float64)
    u = np.einsum("bft,bft->bt", zf, cf)
    pos_denom = np.maximum(
        np.sqrt((zf * zf).sum(axis=1) * (cf * cf).sum(axis=1)), EPS
    )
    pos = (u / pos_denom) * 2.0

    logits = np.concatenate([pos[:, :, None], neg], axis=2).astype(np.float32)
    out = logits.reshape(B * T, K + 1)
    if _trace:
        return out, res
    return out


if __name__ == "__main__":
    rng = np.random.default_rng(0)
    z = rng.standard_normal((B, F, T), dtype=np.float32)
    c = rng.standard_normal((B, F, T + 1), dtype=np.float32)
    ni = rng.integers(0, T - 1, size=(B, T * K)).astype(np.int64)
    out = kernel(z=z, c=c, negative_inds=ni)
    print("out", out.shape, out.dtype, np.isfinite(out).all())
